# revision 4
# baseline (speedup 1.0000x reference)
"""Trainium2 Bass kernel for CausalSelfAttention (GQA + qk-rmsnorm + rope + head gating).

Sharding: 8 cores = 2 (batch) x 4 (kv-head groups). Each core computes the
full attention for one batch element and one kv-head group (4 q heads), plus
its slice of the output projection; partial projection outputs are summed on
the host (bf16 partials, fp32 sum).

Per-core on-device pipeline (all matmuls bf16 with fp32 PSUM accumulation):
  A) fused QKV+gate projection, chunk-outer over token-tile pairs with the
     first two pairs interleaved chunk-major so the PE tracks input DMA
     arrival. Post-processing per tile is pushed off the vector engine:
     - per-head mean-squares via ACT Square+accum_out straight from PSUM,
     - rsqrt as Exp(-0.5*Ln(x)) so every ACT function (Exp/Ln/Square/Copy)
       lives in the single natural_log_exp_and_others table set (no
       table reloads anywhere in the kernel),
     - rms-scale (and q_gain) fused into the ACT PSUM->SBUF bf16 copies via
       the per-partition scale operand,
     - rope as 3 DVE tensor_tensor ops on the combined [q|k] tile using
       packed [cos|cos] / [sin|-sin] bf16 tables and a negative-stride
       "swap halves" access pattern (bf16 2x DVE mode).
     DMA-transpose q,k into head-dim-major layout.
  B) flash-style causal attention per head in S^T layout:
     S^T = K @ Q^T, P = exp(S/sqrt(d)) (no max subtraction: |logits| <= 11.3),
     with S tiles emitted in 2-tile PSUM groups and ONE exp per group
     (halves the ACT instruction overhead on the softmax-critical path),
     diagonal-block masking on DVE, Y = P @ [V | 1] (ones column gives the
     softmax denominator for free). S groups are emitted one group ahead of
     the P@V matmuls. Per-token normalize fuses the sigmoid gate as
     1/(denom*(1+exp(-glog))).
  C) output projection partial: out = y @ Wproj_slice^T, stored bf16.
     C(qc-1) tile blocks are interleaved between B(qc) head blocks.
  PSUM is hand-placed on the 8 banks (no pool rotation) so phase B's first
  S matmuls reuse banks freed by phase A's second-to-last pair instead of
  serializing on the last pair's post-processing.
"""

import numpy as np
import ml_dtypes
from contextlib import ExitStack

import concourse.bass as bass
import concourse.bacc as bacc
import concourse.mybir as mybir
import concourse.tile as tile
from concourse.bass_utils import run_bass_kernel_spmd

BF16 = mybir.dt.bfloat16
F32 = mybir.dt.float32
NPBF = ml_dtypes.bfloat16

B, T, D = 2, 2048, 2048
H, HKV, HD = 16, 4, 128
HALF = HD // 2
NHEAD = H // HKV          # q heads per core (group)
NT = T // 128             # 16 token tiles
NCHUNK = D // 128         # 16 contraction chunks
NQKV = NHEAD * HD + HD + HD + NHEAD   # 512 q + 128 k + 128 v + 4 gate = 772
ROPE_BASE = 10000.0
EPS = float(np.finfo(np.float32).eps)
SM_SCALE = 1.0 / float(np.sqrt(HD))

_CACHE = {}

AF = mybir.ActivationFunctionType


def _ap3(t, d1, d2):
    """View a 2D [128, d1*d2] AP slice as [128, d1, d2]."""
    return bass.AP(tensor=t.tensor, offset=t.offset,
                   ap=[t.ap[0], [d2, d1], [1, d2]])


def _bcast_mid(t, n):
    """[128, a, b] -> [128, n(broadcast), a*b] ... actually broadcast a new
    middle dim of size n over a 2D [128, m] AP: result [128, n, m]."""
    return bass.AP(tensor=t.tensor, offset=t.offset,
                   ap=[t.ap[0], [0, n], t.ap[1]])


def _build_program():
    nc = bacc.Bacc("TRN2", target_bir_lowering=False, debug=False,
                   enable_asserts=False, num_devices=8)

    xT_d = nc.dram_tensor("xT", [D, T], BF16, kind="ExternalInput").ap()
    wqkvg_d = nc.dram_tensor("wqkvg", [D, NQKV], BF16, kind="ExternalInput").ap()
    wproj_d = nc.dram_tensor("wproj", [NHEAD * HD, D], BF16, kind="ExternalInput").ap()
    cc_d = nc.dram_tensor("ccd", [128, NT * HD], BF16, kind="ExternalInput").ap()
    ssn_d = nc.dram_tensor("ssnd", [128, NT * HD], BF16, kind="ExternalInput").ap()
    qgain_d = nc.dram_tensor("qgain", [128, NHEAD], F32, kind="ExternalInput").ap()
    gateb_d = nc.dram_tensor("gateb", [128, NHEAD], F32, kind="ExternalInput").ap()
    mask_d = nc.dram_tensor("mask", [128, 128], BF16, kind="ExternalInput").ap()
    out_d = nc.dram_tensor("out", [T, D], BF16, kind="ExternalOutput").ap()

    with tile.TileContext(nc) as tc, ExitStack() as ctx:
        consts = ctx.enter_context(tc.tile_pool(name="consts", bufs=1))

        # ---- input DMAs: x/w chunk pairs interleaved across the two HWDGE
        # queues; small constants ride behind chunk 2.
        xT_sb = consts.tile([128, NCHUNK, T], BF16)
        wqkvg_sb = consts.tile([128, NCHUNK, NQKV], BF16)
        cc_sb = consts.tile([128, NT, HD], BF16)    # [cos | cos]
        ssn_sb = consts.tile([128, NT, HD], BF16)   # [sin | -sin]
        qgain_sb = consts.tile([128, NHEAD], F32)
        gateb_sb = consts.tile([128, NHEAD], F32)
        mask_sb = consts.tile([128, 128], BF16)
        eps_sb = consts.tile([128, 1], F32)
        nc.vector.memset(eps_sb, EPS)

        def load_chunk(c):
            qx = nc.sync if c % 2 == 0 else nc.scalar
            qw = nc.scalar if c % 2 == 0 else nc.sync
            qx.dma_start(out=xT_sb[:, c, :], in_=xT_d[c * 128:(c + 1) * 128, :])
            qw.dma_start(out=wqkvg_sb[:, c, :],
                         in_=wqkvg_d[c * 128:(c + 1) * 128, :])

        for c in range(3):
            load_chunk(c)
        nc.scalar.dma_start(out=cc_sb.rearrange("p a b -> p (a b)"), in_=cc_d)
        nc.sync.dma_start(out=ssn_sb.rearrange("p a b -> p (a b)"), in_=ssn_d)
        nc.sync.dma_start(out=qgain_sb, in_=qgain_d)
        nc.sync.dma_start(out=gateb_sb, in_=gateb_d)
        nc.scalar.dma_start(out=mask_sb, in_=mask_d)
        for c in range(3, NCHUNK):
            load_chunk(c)
        wproj_sb = consts.tile([128, NHEAD, D], BF16)

        qT_sb = consts.tile([128, NHEAD, T], BF16)   # head-dim-major q
        kT_sb = consts.tile([128, T], BF16)          # head-dim-major k
        v_sb = consts.tile([128, NT, HD + 1], BF16)  # [v | ones] per ki tile
        nc.vector.memset(v_sb[:, :, HD:HD + 1], 1.0)
        yT_sb = consts.tile([128, NHEAD, T], BF16)   # head-dim-major gated y
        glog_all = consts.tile([128, NT, NHEAD], F32)
        egp1_all = consts.tile([128, NT, NHEAD], F32)  # 1 + exp(-glog)

        # ---- the single manually-placed PSUM allocation (all 8 banks)
        psum = ctx.enter_context(tc.tile_pool(name="psum", bufs=1,
                                              space="PSUM"))
        psA = psum.tile([128, 8, 512], F32)

        # =========== Phase A: QKV + gate, rms stats via ACT, rope ============
        a_sb = ctx.enter_context(tc.tile_pool(name="phA", bufs=2))
        junk_sb = ctx.enter_context(tc.tile_pool(name="junk", bufs=1))
        junk = junk_sb.tile([128, 128], BF16)

        def emit_tile_post(tt, qa, qb):
            """Post-processing for one 128-token tile.
            qa = psA slice [128, 512] (4 q heads), qb = [128, 512] (260 used:
            k 0:128, v 128:256, gate 256:260)."""
            ts = slice(tt * 128, (tt + 1) * 128)
            # per-head mean squares (pre-rope == post-rope: rope is a
            # norm-preserving rotation) on ACT, straight from PSUM
            msq = a_sb.tile([128, NHEAD + 1], F32, tag="msq")
            for h in range(NHEAD):
                nc.scalar.activation(out=junk, in_=qa[:, h * HD:(h + 1) * HD],
                                     func=AF.Square,
                                     accum_out=msq[:, h:h + 1])
            nc.scalar.activation(out=junk, in_=qb[:, 0:HD], func=AF.Square,
                                 accum_out=msq[:, NHEAD:NHEAD + 1])
            # rinv = (ms/HD + eps)^-0.5 = Exp(-0.5 * Ln(.)) -- stays in the
            # natural_log_exp_and_others table set
            lnms = a_sb.tile([128, NHEAD + 1], F32, tag="lnms")
            nc.scalar.activation(out=lnms, in_=msq, func=AF.Ln,
                                 scale=1.0 / HD, bias=eps_sb)
            rinv = a_sb.tile([128, NHEAD + 1], F32, tag="rinv")
            nc.scalar.activation(out=rinv, in_=lnms, func=AF.Exp, scale=-0.5)
            rq = a_sb.tile([128, NHEAD], F32, tag="rq")
            nc.vector.tensor_mul(rq, rinv[:, 0:NHEAD], qgain_sb)

            # gate logits
            nc.vector.tensor_add(glog_all[:, tt, :], qb[:, 256:260], gateb_sb)

            # scaled bf16 staging copies (rms-scale fused into the copy)
            qk = a_sb.tile([128, NHEAD + 1, HD], BF16, tag="qk")
            for h in range(NHEAD):
                nc.scalar.activation(out=qk[:, h, :],
                                     in_=qa[:, h * HD:(h + 1) * HD],
                                     func=AF.Copy, scale=rq[:, h:h + 1])
            nc.scalar.activation(out=qk[:, NHEAD, :], in_=qb[:, 0:HD],
                                 func=AF.Copy,
                                 scale=rinv[:, NHEAD:NHEAD + 1])
            nc.scalar.activation(out=v_sb[:, tt, 0:HD], in_=qb[:, 128:256],
                                 func=AF.Copy)

            # rope on scaled values: q' = x*cc + swap(x)*ssn
            cc_t = _bcast_mid(cc_sb[:, tt, :], NHEAD + 1)
            ssn_s = ssn_sb[:, tt, :]
            ssn_t4 = bass.AP(tensor=ssn_s.tensor, offset=ssn_s.offset,
                             ap=[ssn_s.ap[0], [0, NHEAD + 1], [HALF, 2],
                                 [1, HALF]])
            swap = bass.AP(tensor=qk.tensor, offset=qk.offset + HALF,
                           ap=[qk.ap[0], [HD, NHEAD + 1], [-HALF, 2],
                               [1, HALF]])
            u1 = a_sb.tile([128, NHEAD + 1, HD], BF16, tag="u1")
            u2 = a_sb.tile([128, NHEAD + 1, HD], BF16, tag="u2")
            u2_4d = bass.AP(tensor=u2.tensor, offset=u2.offset,
                            ap=[u2.ap[0], [HD, NHEAD + 1], [HALF, 2],
                                [1, HALF]])
            nc.vector.tensor_mul(u1, qk, cc_t)
            nc.vector.tensor_mul(u2_4d, swap, ssn_t4)
            qkr = a_sb.tile([128, NHEAD + 1, HD], BF16, tag="qkr")
            nc.vector.tensor_add(qkr, u1, u2)

            nc.sync.dma_start_transpose(out=qT_sb[:, :, ts],
                                        in_=qkr[:, 0:NHEAD, :])
            nc.sync.dma_start_transpose(out=kT_sb[:, ts], in_=qkr[:, NHEAD, :])

        NPAIR = NT // 2

        def pair_banks(p):
            half = 4 * (p % 2)
            qa = psA[:, half:half + 2, :]     # [128, 2, 512]
            qb = psA[:, half + 2:half + 4, :]
            return qa, qb

        qa0, qb0 = pair_banks(0)
        qa1, qb1 = pair_banks(1)
        # pairs 0 and 1 interleaved chunk-major: the PE tracks DMA arrival
        for c in range(NCHUNK):
            for (qa, qb, pr) in ((qa0, qb0, 0), (qa1, qb1, 1)):
                for ti in range(2):
                    tt = pr * 2 + ti
                    lhs = xT_sb[:, c, tt * 128:(tt + 1) * 128]
                    nc.tensor.matmul(qa[:, ti, :], lhsT=lhs,
                                     rhs=wqkvg_sb[:, c, 0:512],
                                     start=(c == 0), stop=(c == NCHUNK - 1))
                    nc.tensor.matmul(qb[:, ti, 0:NQKV - 512], lhsT=lhs,
                                     rhs=wqkvg_sb[:, c, 512:NQKV],
                                     start=(c == 0), stop=(c == NCHUNK - 1))
        for tt in range(4):
            qa, qb = pair_banks(tt // 2)
            emit_tile_post(tt, qa[:, tt % 2, :], qb[:, tt % 2, :])

        # pairs 2..7: ti-major so the first tile's post can free banks while
        # the second streams
        for pr in range(2, NPAIR):
            qa, qb = pair_banks(pr)
            for ti in range(2):
                tt = pr * 2 + ti
                for c in range(NCHUNK):
                    lhs = xT_sb[:, c, tt * 128:(tt + 1) * 128]
                    nc.tensor.matmul(qa[:, ti, :], lhsT=lhs,
                                     rhs=wqkvg_sb[:, c, 0:512],
                                     start=(c == 0), stop=(c == NCHUNK - 1))
                    nc.tensor.matmul(qb[:, ti, 0:NQKV - 512], lhsT=lhs,
                                     rhs=wqkvg_sb[:, c, 512:NQKV],
                                     start=(c == 0), stop=(c == NCHUNK - 1))
                emit_tile_post(tt, qa[:, ti, :], qb[:, ti, :])
            if pr == 2:
                # wproj: needed only by phase C; keep it off the phase-A
                # critical DMA window
                for h in range(NHEAD):
                    nc.sync.dma_start(out=wproj_sb[:, h, :],
                                      in_=wproj_d[h * 128:(h + 1) * 128, :])

        # =========== Phase B + C: attention, projection =======================
        b_sb = ctx.enter_context(tc.tile_pool(name="phB", bufs=3))
        c_sb = ctx.enter_context(tc.tile_pool(name="phC", bufs=3))

        def emit_C_block(qc, qs, ob):
            tt = qc * 4 + qs
            ts = slice(tt * 128, (tt + 1) * 128)
            for nch in range(4):
                o_ps = psA[:, ob[nch % 2], :]
                for h in range(NHEAD):
                    nc.tensor.matmul(o_ps, lhsT=yT_sb[:, h, ts],
                                     rhs=wproj_sb[:, h, nch * 512:(nch + 1) * 512],
                                     start=(h == 0), stop=(h == NHEAD - 1))
                o_st = c_sb.tile([128, 512], BF16, tag="o_st")
                # vector while exps are still streaming (scalar must stay
                # exp-pure); scalar for the exp-free final-qc tail
                if qc == 3:
                    nc.scalar.copy(out=o_st, in_=o_ps)
                else:
                    nc.vector.tensor_copy(out=o_st, in_=o_ps)
                nc.sync.dma_start(out=out_d[ts, nch * 512:(nch + 1) * 512],
                                  in_=o_st)

        # S bank rotation state: full/diag-a units alternate bank pairs
        # (0,1)/(2,3); diag-b unit lives in bank 7 (time-disjoint from the
        # C-block's second buffer); y01/y23 in banks 5/6; C o in {4, 7}.
        sflip = [0]

        def unit_list(qc):
            """Units for one (h, qc): list of (kis, widths, kind).
            kind: 'pair' (2 banks) or 'diag1' (single bank 7)."""
            units = []
            for k0 in range(0, 4 * qc, 2):
                units.append(((k0, k0 + 1), (512, 512), 'pair'))
            d = 4 * qc
            units.append(((d, d + 1), (512, 384), 'pair'))
            units.append(((d + 2, d + 3), (256, 128), 'diag1'))
            return units

        y01 = _ap3(psA[:, 5, 0:2 * (HD + 1)], 2, HD + 1)
        y23 = _ap3(psA[:, 6, 0:2 * (HD + 1)], 2, HD + 1)

        for qc in range(4):
            # gate for this qc's tiles: 1 + exp(-glog) (Exp table resident)
            eg = egp1_all[:, qc * 4:(qc + 1) * 4, :]
            eg_f = eg.rearrange("p a b -> p (a b)")
            nc.scalar.activation(
                out=eg_f,
                in_=glog_all[:, qc * 4:(qc + 1) * 4, :].rearrange(
                    "p a b -> p (a b)"),
                func=AF.Exp, scale=-1.0)
            nc.vector.tensor_scalar_add(eg_f, eg_f, 1.0)

            units = unit_list(qc)

            def emit_S_unit(h, u):
                """S matmuls for one unit + one batched exp (+ masks).
                Returns (p2, col_of_ki)."""
                kis, widths, kind = u
                if kind == 'pair':
                    base = sflip[0]
                    sflip[0] ^= 2
                    s_ap = psA[:, base, :]  # flat view over 2 banks
                    s_flat = bass.AP(tensor=s_ap.tensor, offset=s_ap.offset,
                                     ap=[s_ap.ap[0], [1, 1024]])
                    offs = (0, 512)
                else:
                    s_flat = psA[:, 7, 0:384]
                    offs = (0, 256)
                cols = {}
                for j, ki in enumerate(kis):
                    w = widths[j]
                    m = ki - 4 * qc
                    q_lo = qc * 512 + 128 * max(m, 0)
                    nc.tensor.matmul(s_flat[:, offs[j]:offs[j] + w],
                                     lhsT=kT_sb[:, ki * 128:(ki + 1) * 128],
                                     rhs=qT_sb[:, h, q_lo:(qc + 1) * 512],
                                     start=True, stop=True)
                    cols[ki] = offs[j]
                wtot = offs[1] + widths[1]
                p2 = b_sb.tile([128, 1024], BF16, tag="p2")
                nc.scalar.activation(out=p2[:, 0:wtot], in_=s_flat[:, 0:wtot],
                                     func=AF.Exp, scale=SM_SCALE)
                for j, ki in enumerate(kis):
                    if ki - 4 * qc >= 0:
                        nc.vector.tensor_mul(
                            p2[:, cols[ki]:cols[ki] + 128],
                            p2[:, cols[ki]:cols[ki] + 128], mask_sb)
                return p2, cols

            def emit_PV(h, u, p2, cols):
                kis, widths, kind = u
                for j, ki in enumerate(kis):
                    m = ki - 4 * qc
                    for qs in range(max(m, 0), 4):
                        ytile = y01 if qs < 2 else y23
                        pcol = cols[ki] + (qs - max(m, 0)) * 128
                        nc.tensor.matmul(
                            ytile[:, qs % 2, :],
                            lhsT=p2[:, pcol:pcol + 128],
                            rhs=v_sb[:, ki, :],
                            start=(ki == 0 and qs % 2 == 0),
                            stop=(ki == 4 * qc + qs and qs % 2 == 1))

            for h in range(NHEAD):
                nu = len(units)
                cur = emit_S_unit(h, units[0])
                for ui in range(nu):
                    nxt = emit_S_unit(h, units[ui + 1]) if ui + 1 < nu else None
                    emit_PV(h, units[ui], *cur)
                    cur = nxt
                # normalize + fused sigmoid gate, batched per qs-pair
                y_stage = b_sb.tile([128, 4, HD], BF16, tag="y_stage")
                for half in range(2):
                    ytile = y01 if half == 0 else y23
                    tt0 = qc * 4 + 2 * half
                    den = b_sb.tile([128, 2, 1], F32, tag="den")
                    nc.vector.tensor_mul(den,
                                         egp1_all[:, tt0:tt0 + 2, h:h + 1],
                                         ytile[:, :, HD:HD + 1])
                    sc = b_sb.tile([128, 2, 1], F32, tag="sc")
                    nc.vector.reciprocal(sc, den)
                    sc_b = bass.AP(tensor=sc.tensor, offset=sc.offset,
                                   ap=[sc.ap[0], sc.ap[1], [0, HD]])
                    nc.vector.tensor_mul(y_stage[:, 2 * half:2 * half + 2, :],
                                         ytile[:, :, 0:HD], sc_b)
                yreg = yT_sb[:, h, qc * 512:(qc + 1) * 512]
                y3d = bass.AP(tensor=yreg.tensor, offset=yreg.offset,
                              ap=[yreg.ap[0], [128, 4], [1, 128]])
                nc.sync.dma_start_transpose(out=y3d, in_=y_stage)

                # C for the previous qc rides between B head blocks
                if qc >= 1:
                    emit_C_block(qc - 1, h, (4, 7))

        for qs in range(4):
            emit_C_block(3, qs, (4, 7))

    nc.compile()
    return nc


def _get_program():
    if "nc" not in _CACHE:
        _CACHE["nc"] = _build_program()
    return _CACHE["nc"]


def _host_prep(x, Wq, Wk, Wv, Wproj, q_gain, gate_w, gate_b):
    """Build the 8 per-core input maps."""
    f = np.float32
    x = np.asarray(x, f)
    WqT = np.asarray(Wq, f).T.astype(NPBF)       # [D, 2048]
    WkT = np.asarray(Wk, f).T.astype(NPBF)       # [D, 512]
    WvT = np.asarray(Wv, f).T.astype(NPBF)
    WpT = np.ascontiguousarray(np.asarray(Wproj, f).T.astype(NPBF))  # [D, D]
    gwT = np.asarray(gate_w, f).T.astype(NPBF)   # [D, 16]
    q_gain = np.asarray(q_gain, f)
    gate_b = np.asarray(gate_b, f)

    inv_freq = 1.0 / (ROPE_BASE ** (np.arange(0, HD, 2, dtype=f) / HD))
    tpos = np.arange(T, dtype=f)
    freqs = np.outer(tpos, inv_freq)             # [T, HALF]
    cos = np.cos(freqs).astype(f)
    sin = np.sin(freqs).astype(f)
    # packed rope tables, device layout [128 partitions, NT tiles, HD]:
    # cc = [cos | cos], ssn = [sin | -sin]
    cc = np.concatenate([cos, cos], axis=1)      # [T, HD]
    ssn = np.concatenate([sin, -sin], axis=1)
    cc = np.ascontiguousarray(
        cc.reshape(NT, 128, HD).transpose(1, 0, 2).astype(NPBF)
    ).reshape(128, NT * HD)
    ssn = np.ascontiguousarray(
        ssn.reshape(NT, 128, HD).transpose(1, 0, 2).astype(NPBF)
    ).reshape(128, NT * HD)

    kloc = np.arange(128)[:, None]
    qloc = np.arange(128)[None, :]
    mask = (qloc >= kloc).astype(NPBF)           # [128, 128]

    xT = [np.ascontiguousarray(x[b].T).astype(NPBF) for b in range(B)]

    in_maps = []
    for core in range(8):
        b, g = divmod(core, 4)
        wqkvg = np.concatenate([
            WqT[:, 512 * g:512 * (g + 1)],
            WkT[:, 128 * g:128 * (g + 1)],
            WvT[:, 128 * g:128 * (g + 1)],
            gwT[:, NHEAD * g:NHEAD * (g + 1)],
        ], axis=1)                               # [D, 772]
        in_maps.append({
            "xT": xT[b],
            "wqkvg": np.ascontiguousarray(wqkvg),
            "wproj": np.ascontiguousarray(WpT[512 * g:512 * (g + 1), :]),
            "ccd": cc,
            "ssnd": ssn,
            "qgain": np.ascontiguousarray(np.broadcast_to(
                q_gain[NHEAD * g:NHEAD * (g + 1)][None, :], (128, NHEAD))),
            "gateb": np.ascontiguousarray(np.broadcast_to(
                gate_b[NHEAD * g:NHEAD * (g + 1)][None, :], (128, NHEAD))),
            "mask": mask,
        })
    return in_maps


def kernel(**inputs):
    nc = _get_program()
    in_maps = _host_prep(**inputs)
    res = run_bass_kernel_spmd(nc, in_maps, list(range(8)))
    parts = [r["out"] for r in res.results]
    out = np.empty((B, T, D), np.float32)
    for b in range(B):
        out[b] = (parts[4 * b].astype(np.float32)
                  + parts[4 * b + 1].astype(np.float32)
                  + parts[4 * b + 2].astype(np.float32)
                  + parts[4 * b + 3].astype(np.float32))
    return out


# revision 7
# speedup vs baseline: 1.1550x; 1.1550x over previous
"""Trainium2 Bass kernel for CausalSelfAttention (GQA + qk-rmsnorm + rope + head gating).

Sharding: 8 cores = 2 (batch) x 4 (kv-head groups). Each core computes the
full attention for one batch element and one kv-head group (4 q heads), plus
its slice of the output projection; partial projection outputs are summed on
the host (bf16 partials, fp32 sum).

Per-core on-device pipeline (all matmuls bf16 with fp32 PSUM accumulation):
  A) fused QKV+gate projection, chunk-outer over token-tile pairs with the
     first two pairs interleaved chunk-major so the PE tracks input DMA
     arrival. Post-processing per tile is pushed off the vector engine:
     - per-head mean-squares via ACT Square+accum_out straight from PSUM,
     - rsqrt as Exp(-0.5*Ln(x)) so every ACT function (Exp/Ln/Square/Copy)
       lives in the single natural_log_exp_and_others table set (no
       table reloads anywhere in the kernel),
     - rms-scale (and q_gain) fused into the ACT PSUM->SBUF bf16 copies via
       the per-partition scale operand,
     - rope as 3 DVE tensor_tensor ops on the combined [q|k] tile using
       packed [cos|cos] / [sin|-sin] bf16 tables and a negative-stride
       "swap halves" access pattern (bf16 2x DVE mode).
     DMA-transpose q,k into head-dim-major layout.
  B) flash-style causal attention per head in S^T layout:
     S^T = K @ Q^T, P = exp(S/sqrt(d)) (no max subtraction: |logits| <= 11.3),
     with S tiles emitted in 2-tile PSUM groups and ONE exp per group
     (halves the ACT instruction overhead on the softmax-critical path),
     diagonal-block masking on DVE, Y = P @ [V | 1] (ones column gives the
     softmax denominator for free). S groups are emitted one group ahead of
     the P@V matmuls. Per-token normalize fuses the sigmoid gate as
     1/(denom*(1+exp(-glog))).
  C) output projection partial: out = y @ Wproj_slice^T, stored bf16.
     C(qc-1) tile blocks are interleaved between B(qc) head blocks.
  PSUM is hand-placed on the 8 banks (no pool rotation) so phase B's first
  S matmuls reuse banks freed by phase A's second-to-last pair instead of
  serializing on the last pair's post-processing.
"""

import numpy as np
import ml_dtypes
from contextlib import ExitStack

import concourse.bass as bass
import concourse.bacc as bacc
import concourse.mybir as mybir
import concourse.tile as tile
from concourse.bass_utils import run_bass_kernel_spmd

BF16 = mybir.dt.bfloat16
F32 = mybir.dt.float32
NPBF = ml_dtypes.bfloat16

B, T, D = 2, 2048, 2048
H, HKV, HD = 16, 4, 128
HALF = HD // 2
NHEAD = H // HKV          # q heads per core (group)
NT = T // 128             # 16 token tiles
NCHUNK = D // 128         # 16 contraction chunks
NQKV = NHEAD * HD + HD + HD + NHEAD   # 512 q + 128 k + 128 v + 4 gate = 772
ROPE_BASE = 10000.0
EPS = float(np.finfo(np.float32).eps)
SM_SCALE = 1.0 / float(np.sqrt(HD))

_CACHE = {}

AF = mybir.ActivationFunctionType


def _ap3(t, d1, d2):
    """View a 2D [128, d1*d2] AP slice as [128, d1, d2]."""
    return bass.AP(tensor=t.tensor, offset=t.offset,
                   ap=[t.ap[0], [d2, d1], [1, d2]])


def _bcast_mid(t, n):
    """[128, a, b] -> [128, n(broadcast), a*b] ... actually broadcast a new
    middle dim of size n over a 2D [128, m] AP: result [128, n, m]."""
    return bass.AP(tensor=t.tensor, offset=t.offset,
                   ap=[t.ap[0], [0, n], t.ap[1]])


def _build_program():
    nc = bacc.Bacc("TRN2", target_bir_lowering=False, debug=False,
                   enable_asserts=False, num_devices=8)

    xT_d = nc.dram_tensor("xT", [D, T], BF16, kind="ExternalInput").ap()
    wqkvg_d = nc.dram_tensor("wqkvg", [D, NQKV], BF16, kind="ExternalInput").ap()
    wproj_d = nc.dram_tensor("wproj", [NHEAD * HD, D], BF16, kind="ExternalInput").ap()
    cc_d = nc.dram_tensor("ccd", [128, NT * HD], BF16, kind="ExternalInput").ap()
    ssn_d = nc.dram_tensor("ssnd", [128, NT * HD], BF16, kind="ExternalInput").ap()
    qgain_d = nc.dram_tensor("qgain", [128, NHEAD], F32, kind="ExternalInput").ap()
    gateb_d = nc.dram_tensor("gateb", [128, NHEAD], F32, kind="ExternalInput").ap()
    mask_d = nc.dram_tensor("mask", [128, 128], BF16, kind="ExternalInput").ap()
    out_d = nc.dram_tensor("out", [T, D], BF16, kind="ExternalOutput").ap()

    with tile.TileContext(nc) as tc, ExitStack() as ctx:
        consts = ctx.enter_context(tc.tile_pool(name="consts", bufs=1))

        # ---- input DMAs: x/w chunk pairs interleaved across the two HWDGE
        # queues; small constants ride behind chunk 2.
        xT_sb = consts.tile([128, NCHUNK, T], BF16)
        wqkvg_sb = consts.tile([128, NCHUNK, NQKV], BF16)
        cc_sb = consts.tile([128, NT, HD], BF16)    # [cos | cos]
        ssn_sb = consts.tile([128, NT, HD], BF16)   # [sin | -sin]
        qgain_sb = consts.tile([128, NHEAD], F32)
        gateb_sb = consts.tile([128, NHEAD], F32)
        mask_sb = consts.tile([128, 128], BF16)
        eps_sb = consts.tile([128, 1], F32)
        nc.vector.memset(eps_sb, EPS)

        def load_chunk(c):
            qx = nc.sync if c % 2 == 0 else nc.scalar
            qw = nc.scalar if c % 2 == 0 else nc.sync
            qx.dma_start(out=xT_sb[:, c, :], in_=xT_d[c * 128:(c + 1) * 128, :])
            qw.dma_start(out=wqkvg_sb[:, c, :],
                         in_=wqkvg_d[c * 128:(c + 1) * 128, :])

        for c in range(3):
            load_chunk(c)
        nc.scalar.dma_start(out=cc_sb.rearrange("p a b -> p (a b)"), in_=cc_d)
        nc.sync.dma_start(out=ssn_sb.rearrange("p a b -> p (a b)"), in_=ssn_d)
        nc.sync.dma_start(out=qgain_sb, in_=qgain_d)
        nc.sync.dma_start(out=gateb_sb, in_=gateb_d)
        nc.scalar.dma_start(out=mask_sb, in_=mask_d)
        for c in range(3, NCHUNK):
            load_chunk(c)
        wproj_sb = consts.tile([128, NHEAD, D], BF16)

        qT_sb = consts.tile([128, NHEAD, T], BF16)   # head-dim-major q
        kT_sb = consts.tile([128, T], BF16)          # head-dim-major k
        v_sb = consts.tile([128, NT, HD + 1], BF16)  # [v | ones] per ki tile
        nc.vector.memset(v_sb[:, :, HD:HD + 1], 1.0)
        yT_sb = consts.tile([128, NHEAD, T], BF16)   # head-dim-major gated y
        glog_all = consts.tile([128, NT, NHEAD], F32)
        egp1_all = consts.tile([128, NT, NHEAD], F32)  # 1 + exp(-glog)

        # ---- the single manually-placed PSUM allocation (all 8 banks)
        psum = ctx.enter_context(tc.tile_pool(name="psum", bufs=1,
                                              space="PSUM"))
        psA = psum.tile([128, 8, 512], F32)

        # =========== Phase A: QKV + gate, rms stats via ACT, rope ============
        a_sb = ctx.enter_context(tc.tile_pool(name="phA", bufs=2))
        junk_sb = ctx.enter_context(tc.tile_pool(name="junk", bufs=1))
        junk = junk_sb.tile([128, 128], BF16)

        def emit_tile_post(tt, qa, qb):
            """Post-processing for one 128-token tile.
            qa = psA slice [128, 512] (4 q heads), qb = [128, 512] (260 used:
            k 0:128, v 128:256, gate 256:260).
            ACT functions here are Square/Sqrt/Copy only -- all live in the
            sqrt_and_others table set, so phase A does one table load."""
            ts = slice(tt * 128, (tt + 1) * 128)
            # per-head mean squares (pre-rope == post-rope: rope is a
            # norm-preserving rotation) on ACT, straight from PSUM
            msq = a_sb.tile([128, NHEAD + 1], F32, tag="msq")
            for h in range(NHEAD):
                nc.scalar.activation(out=junk, in_=qa[:, h * HD:(h + 1) * HD],
                                     func=AF.Square,
                                     accum_out=msq[:, h:h + 1])
            nc.scalar.activation(out=junk, in_=qb[:, 0:HD], func=AF.Square,
                                 accum_out=msq[:, NHEAD:NHEAD + 1])
            rtmp = a_sb.tile([128, NHEAD + 1], F32, tag="rtmp")
            nc.scalar.activation(out=rtmp, in_=msq, func=AF.Sqrt,
                                 scale=1.0 / HD, bias=eps_sb)
            rinv = a_sb.tile([128, NHEAD + 1], F32, tag="rinv")
            nc.vector.reciprocal(rinv, rtmp)
            rq = a_sb.tile([128, NHEAD], F32, tag="rq")
            nc.vector.tensor_mul(rq, rinv[:, 0:NHEAD], qgain_sb)

            # gate logits
            nc.vector.tensor_add(glog_all[:, tt, :], qb[:, 256:260], gateb_sb)

            # unscaled bf16 staging copies (batched; scale lands post-rope)
            qk = a_sb.tile([128, NHEAD + 1, HD], BF16, tag="qk")
            nc.scalar.activation(
                out=qk[:, 0:NHEAD, :].rearrange("p a b -> p (a b)"),
                in_=qa, func=AF.Copy)
            nc.scalar.activation(out=qk[:, NHEAD, :], in_=qb[:, 0:HD],
                                 func=AF.Copy)
            nc.scalar.activation(out=v_sb[:, tt, 0:HD], in_=qb[:, 128:256],
                                 func=AF.Copy)

            # rope on raw values: q' = x*cc + swap(x)*ssn
            cc_t = _bcast_mid(cc_sb[:, tt, :], NHEAD + 1)
            ssn_s = ssn_sb[:, tt, :]
            ssn_t4 = bass.AP(tensor=ssn_s.tensor, offset=ssn_s.offset,
                             ap=[ssn_s.ap[0], [0, NHEAD + 1], [HALF, 2],
                                 [1, HALF]])
            swap = bass.AP(tensor=qk.tensor, offset=qk.offset + HALF,
                           ap=[qk.ap[0], [HD, NHEAD + 1], [-HALF, 2],
                               [1, HALF]])
            u1 = a_sb.tile([128, NHEAD + 1, HD], BF16, tag="u1")
            u2 = a_sb.tile([128, NHEAD + 1, HD], BF16, tag="u2")
            u2_4d = bass.AP(tensor=u2.tensor, offset=u2.offset,
                            ap=[u2.ap[0], [HD, NHEAD + 1], [HALF, 2],
                                [1, HALF]])
            nc.vector.tensor_mul(u1, qk, cc_t)
            nc.vector.tensor_mul(u2_4d, swap, ssn_t4)
            qkr = a_sb.tile([128, NHEAD + 1, HD], BF16, tag="qkr")
            nc.vector.tensor_add(qkr, u1, u2)
            # rms-scale (and q_gain), in place (commutes with the rotation)
            rq_b = bass.AP(tensor=rq.tensor, offset=rq.offset,
                           ap=[rq.ap[0], rq.ap[1], [0, HD]])
            nc.vector.tensor_mul(qkr[:, 0:NHEAD, :], qkr[:, 0:NHEAD, :], rq_b)
            nc.vector.tensor_scalar_mul(qkr[:, NHEAD, :], qkr[:, NHEAD, :],
                                        rinv[:, NHEAD:NHEAD + 1])

            nc.sync.dma_start_transpose(out=qT_sb[:, :, ts],
                                        in_=qkr[:, 0:NHEAD, :])
            nc.sync.dma_start_transpose(out=kT_sb[:, ts], in_=qkr[:, NHEAD, :])

        NPAIR = NT // 2

        def pair_banks(p):
            half = 4 * (p % 2)
            qa = psA[:, half:half + 2, :]     # [128, 2, 512]
            qb = psA[:, half + 2:half + 4, :]
            return qa, qb

        qa0, qb0 = pair_banks(0)
        qa1, qb1 = pair_banks(1)
        # pairs 0 and 1 interleaved chunk-major: the PE tracks DMA arrival
        for c in range(NCHUNK):
            for (qa, qb, pr) in ((qa0, qb0, 0), (qa1, qb1, 1)):
                for ti in range(2):
                    tt = pr * 2 + ti
                    lhs = xT_sb[:, c, tt * 128:(tt + 1) * 128]
                    nc.tensor.matmul(qa[:, ti, :], lhsT=lhs,
                                     rhs=wqkvg_sb[:, c, 0:512],
                                     start=(c == 0), stop=(c == NCHUNK - 1))
                    nc.tensor.matmul(qb[:, ti, 0:NQKV - 512], lhsT=lhs,
                                     rhs=wqkvg_sb[:, c, 512:NQKV],
                                     start=(c == 0), stop=(c == NCHUNK - 1))
        for tt in range(4):
            qa, qb = pair_banks(tt // 2)
            emit_tile_post(tt, qa[:, tt % 2, :], qb[:, tt % 2, :])

        # pairs 2..7: ti-major so the first tile's post can free banks while
        # the second streams
        for pr in range(2, NPAIR):
            qa, qb = pair_banks(pr)
            for ti in range(2):
                tt = pr * 2 + ti
                for c in range(NCHUNK):
                    lhs = xT_sb[:, c, tt * 128:(tt + 1) * 128]
                    nc.tensor.matmul(qa[:, ti, :], lhsT=lhs,
                                     rhs=wqkvg_sb[:, c, 0:512],
                                     start=(c == 0), stop=(c == NCHUNK - 1))
                    nc.tensor.matmul(qb[:, ti, 0:NQKV - 512], lhsT=lhs,
                                     rhs=wqkvg_sb[:, c, 512:NQKV],
                                     start=(c == 0), stop=(c == NCHUNK - 1))
                emit_tile_post(tt, qa[:, ti, :], qb[:, ti, :])
            if pr == 2:
                # wproj: needed only by phase C; keep it off the phase-A
                # critical DMA window
                for h in range(NHEAD):
                    nc.sync.dma_start(out=wproj_sb[:, h, :],
                                      in_=wproj_d[h * 128:(h + 1) * 128, :])

        # =========== Phase B + C: attention, projection =======================
        b_sb = ctx.enter_context(tc.tile_pool(name="phB", bufs=3))
        c_sb = ctx.enter_context(tc.tile_pool(name="phC", bufs=3))

        def emit_C_block(qc, qs, ob):
            tt = qc * 4 + qs
            ts = slice(tt * 128, (tt + 1) * 128)
            for nch in range(4):
                o_ps = psA[:, ob[nch % 2], :]
                for h in range(NHEAD):
                    nc.tensor.matmul(o_ps, lhsT=yT_sb[:, h, ts],
                                     rhs=wproj_sb[:, h, nch * 512:(nch + 1) * 512],
                                     start=(h == 0), stop=(h == NHEAD - 1))
                o_st = c_sb.tile([128, 512], BF16, tag="o_st")
                # vector while exps are still streaming (scalar must stay
                # exp-pure); scalar for the exp-free final-qc tail
                if qc == 3:
                    nc.scalar.copy(out=o_st, in_=o_ps)
                else:
                    nc.vector.tensor_copy(out=o_st, in_=o_ps)
                nc.sync.dma_start(out=out_d[ts, nch * 512:(nch + 1) * 512],
                                  in_=o_st)

        # S bank rotation state: full/diag-a units alternate bank pairs
        # (0,1)/(2,3); diag-b unit lives in bank 7 (time-disjoint from the
        # C-block's second buffer); y01/y23 in banks 5/6; C o in {4, 7}.
        sflip = [0]

        def unit_list(qc):
            """Units for one (h, qc): list of (kis, widths, kind).
            kind: 'pair' (2 banks) or 'diag1' (single bank 7)."""
            units = []
            for k0 in range(0, 4 * qc, 2):
                units.append(((k0, k0 + 1), (512, 512), 'pair'))
            d = 4 * qc
            units.append(((d, d + 1), (512, 384), 'pair'))
            units.append(((d + 2, d + 3), (256, 128), 'diag1'))
            return units

        y01 = _ap3(psA[:, 5, 0:2 * (HD + 1)], 2, HD + 1)
        y23 = _ap3(psA[:, 6, 0:2 * (HD + 1)], 2, HD + 1)

        # gates for ALL tiles in one shot: 1 + exp(-glog). Depends on every
        # tile's glog, so the scheduler cannot hoist this Exp into phase A
        # between Sqrts (which would thrash the ACT table set).
        eg_all = egp1_all.rearrange("p a b -> p (a b)")
        nc.scalar.activation(
            out=eg_all, in_=glog_all.rearrange("p a b -> p (a b)"),
            func=AF.Exp, scale=-1.0)
        nc.vector.tensor_scalar_add(eg_all, eg_all, 1.0)

        for qc in range(4):
            units = unit_list(qc)

            def emit_S_unit(h, u):
                """S matmuls for one unit + one batched exp (+ masks).
                Returns (p2, col_of_ki)."""
                kis, widths, kind = u
                if kind == 'pair':
                    base = sflip[0]
                    sflip[0] ^= 2
                    s_ap = psA[:, base, :]  # flat view over 2 banks
                    s_flat = bass.AP(tensor=s_ap.tensor, offset=s_ap.offset,
                                     ap=[s_ap.ap[0], [1, 1024]])
                    offs = (0, 512)
                else:
                    s_flat = psA[:, 7, 0:384]
                    offs = (0, 256)
                cols = {}
                for j, ki in enumerate(kis):
                    w = widths[j]
                    m = ki - 4 * qc
                    q_lo = qc * 512 + 128 * max(m, 0)
                    nc.tensor.matmul(s_flat[:, offs[j]:offs[j] + w],
                                     lhsT=kT_sb[:, ki * 128:(ki + 1) * 128],
                                     rhs=qT_sb[:, h, q_lo:(qc + 1) * 512],
                                     start=True, stop=True)
                    cols[ki] = offs[j]
                wtot = offs[1] + widths[1]
                p2 = b_sb.tile([128, 1024], BF16, tag="p2")
                nc.scalar.activation(out=p2[:, 0:wtot], in_=s_flat[:, 0:wtot],
                                     func=AF.Exp, scale=SM_SCALE)
                for j, ki in enumerate(kis):
                    if ki - 4 * qc >= 0:
                        nc.vector.tensor_mul(
                            p2[:, cols[ki]:cols[ki] + 128],
                            p2[:, cols[ki]:cols[ki] + 128], mask_sb)
                return p2, cols

            def emit_PV(h, u, p2, cols):
                kis, widths, kind = u
                for j, ki in enumerate(kis):
                    m = ki - 4 * qc
                    for qs in range(max(m, 0), 4):
                        ytile = y01 if qs < 2 else y23
                        pcol = cols[ki] + (qs - max(m, 0)) * 128
                        nc.tensor.matmul(
                            ytile[:, qs % 2, :],
                            lhsT=p2[:, pcol:pcol + 128],
                            rhs=v_sb[:, ki, :],
                            start=(ki == 0 and qs % 2 == 0),
                            stop=(ki == 4 * qc + qs and qs % 2 == 1))

            for h in range(NHEAD):
                nu = len(units)
                cur = emit_S_unit(h, units[0])
                for ui in range(nu):
                    nxt = emit_S_unit(h, units[ui + 1]) if ui + 1 < nu else None
                    emit_PV(h, units[ui], *cur)
                    cur = nxt
                # normalize + fused sigmoid gate, batched per qs-pair
                y_stage = b_sb.tile([128, 4, HD], BF16, tag="y_stage")
                for half in range(2):
                    ytile = y01 if half == 0 else y23
                    tt0 = qc * 4 + 2 * half
                    den = b_sb.tile([128, 2, 1], F32, tag="den")
                    nc.vector.tensor_mul(den,
                                         egp1_all[:, tt0:tt0 + 2, h:h + 1],
                                         ytile[:, :, HD:HD + 1])
                    sc = b_sb.tile([128, 2, 1], F32, tag="sc")
                    nc.vector.reciprocal(sc, den)
                    sc_b = bass.AP(tensor=sc.tensor, offset=sc.offset,
                                   ap=[sc.ap[0], sc.ap[1], [0, HD]])
                    nc.vector.tensor_mul(y_stage[:, 2 * half:2 * half + 2, :],
                                         ytile[:, :, 0:HD], sc_b)
                yreg = yT_sb[:, h, qc * 512:(qc + 1) * 512]
                y3d = bass.AP(tensor=yreg.tensor, offset=yreg.offset,
                              ap=[yreg.ap[0], [128, 4], [1, 128]])
                nc.sync.dma_start_transpose(out=y3d, in_=y_stage)

                # C for the previous qc rides between B head blocks
                if qc >= 1:
                    emit_C_block(qc - 1, h, (4, 7))

        for qs in range(4):
            emit_C_block(3, qs, (4, 7))

    nc.compile()
    return nc


def _get_program():
    if "nc" not in _CACHE:
        _CACHE["nc"] = _build_program()
    return _CACHE["nc"]


def _host_prep(x, Wq, Wk, Wv, Wproj, q_gain, gate_w, gate_b):
    """Build the 8 per-core input maps."""
    f = np.float32
    x = np.asarray(x, f)
    WqT = np.asarray(Wq, f).T.astype(NPBF)       # [D, 2048]
    WkT = np.asarray(Wk, f).T.astype(NPBF)       # [D, 512]
    WvT = np.asarray(Wv, f).T.astype(NPBF)
    WpT = np.ascontiguousarray(np.asarray(Wproj, f).T.astype(NPBF))  # [D, D]
    gwT = np.asarray(gate_w, f).T.astype(NPBF)   # [D, 16]
    q_gain = np.asarray(q_gain, f)
    gate_b = np.asarray(gate_b, f)

    inv_freq = 1.0 / (ROPE_BASE ** (np.arange(0, HD, 2, dtype=f) / HD))
    tpos = np.arange(T, dtype=f)
    freqs = np.outer(tpos, inv_freq)             # [T, HALF]
    cos = np.cos(freqs).astype(f)
    sin = np.sin(freqs).astype(f)
    # packed rope tables, device layout [128 partitions, NT tiles, HD]:
    # cc = [cos | cos], ssn = [sin | -sin]
    cc = np.concatenate([cos, cos], axis=1)      # [T, HD]
    ssn = np.concatenate([sin, -sin], axis=1)
    cc = np.ascontiguousarray(
        cc.reshape(NT, 128, HD).transpose(1, 0, 2).astype(NPBF)
    ).reshape(128, NT * HD)
    ssn = np.ascontiguousarray(
        ssn.reshape(NT, 128, HD).transpose(1, 0, 2).astype(NPBF)
    ).reshape(128, NT * HD)

    kloc = np.arange(128)[:, None]
    qloc = np.arange(128)[None, :]
    mask = (qloc >= kloc).astype(NPBF)           # [128, 128]

    xT = [np.ascontiguousarray(x[b].T).astype(NPBF) for b in range(B)]

    in_maps = []
    for core in range(8):
        b, g = divmod(core, 4)
        wqkvg = np.concatenate([
            WqT[:, 512 * g:512 * (g + 1)],
            WkT[:, 128 * g:128 * (g + 1)],
            WvT[:, 128 * g:128 * (g + 1)],
            gwT[:, NHEAD * g:NHEAD * (g + 1)],
        ], axis=1)                               # [D, 772]
        in_maps.append({
            "xT": xT[b],
            "wqkvg": np.ascontiguousarray(wqkvg),
            "wproj": np.ascontiguousarray(WpT[512 * g:512 * (g + 1), :]),
            "ccd": cc,
            "ssnd": ssn,
            "qgain": np.ascontiguousarray(np.broadcast_to(
                q_gain[NHEAD * g:NHEAD * (g + 1)][None, :], (128, NHEAD))),
            "gateb": np.ascontiguousarray(np.broadcast_to(
                gate_b[NHEAD * g:NHEAD * (g + 1)][None, :], (128, NHEAD))),
            "mask": mask,
        })
    return in_maps


def kernel(**inputs):
    nc = _get_program()
    in_maps = _host_prep(**inputs)
    res = run_bass_kernel_spmd(nc, in_maps, list(range(8)))
    parts = [r["out"] for r in res.results]
    out = np.empty((B, T, D), np.float32)
    for b in range(B):
        out[b] = (parts[4 * b].astype(np.float32)
                  + parts[4 * b + 1].astype(np.float32)
                  + parts[4 * b + 2].astype(np.float32)
                  + parts[4 * b + 3].astype(np.float32))
    return out
